# revision 1
# baseline (speedup 1.0000x reference)
"""Trainium2 Bass kernel for nn_DynamicSparseAttention (B=2,S=1024,E=1024,H=16,K=64).

Sharding: 8 cores = 2 batches x 4 head-groups (4 heads each).

Per core: QKV projections for its 4 heads (f32 Q/K since top-k selection is
precision-critical), per-head scores, exact top-64 per query row via 8 rounds
of DVE max8 + in-place match_replace on +128-shifted scores (8th
match_replace masks ranks 57-64). Masked softmax via the complement-exp
trick: pm = (exp(sc*sB) - exp(sc*z_final)) * rden computed as ACT exps (no
mask pass; exact cancellation off the top-64) plus one DVE subtract and one
tensor_scalar multiply -- no Ln (avoids ACT table thrash), no is_ge/stt.
V projection and pair-1 Q/K projection overlap the early heads' scans;
Wo/bv loads issue after phase-A x loads. pm transposed via sync-queue DMA
transposes, AV with V stationary, AllGather per head-pair (first overlaps
remaining compute), and a column-split output projection: each core computes
its 256 of 1024 output columns; the host concatenates.

Notes from profiling (throttle-limited device, ~53% util cap): GpSimd
offload of elementwise work is counterproductive -- each Pool op carries
~2us semaphore overhead and the Q7 cores double as collective-compute
cores, so GpSimd tensor_tensor fights the AllGather. ACT Sign/Ln force
~1.3us activation-table reloads per switch. fp32r matmul is TF32-grade
(rel ~1.5e-4): too coarse for scores (selection needs <~1e-5 noise).
is_transpose matmul with a non-identity moving operand produces garbage.
"""

import sys

if "/opt/trn_rl_repo" not in sys.path:
    sys.path.insert(0, "/opt/trn_rl_repo")

import numpy as np
import ml_dtypes


def _install_ntff_hook_module():
    """bass_utils(trace=True) imports antenv.axon_hooks, which this image's
    read-only antenv lacks; provide it via sys.modules (ctypes into
    libaxon_pjrt.so, same recipe as the boot script)."""
    import types, contextlib, ctypes

    if "antenv.axon_hooks" in sys.modules:
        return
    mod = types.ModuleType("antenv.axon_hooks")
    state = {"hook": None}

    def _make_hook(so_path="/opt/axon/libaxon_pjrt.so"):
        lib = ctypes.CDLL(so_path)
        if not hasattr(lib, "axon_start_nrt_profile"):
            return None
        lib.axon_start_nrt_profile.argtypes = [
            ctypes.POINTER(ctypes.c_int64), ctypes.c_size_t]
        lib.axon_start_nrt_profile.restype = ctypes.c_int64
        lib.axon_stop_nrt_profile.argtypes = [ctypes.c_char_p]
        lib.axon_stop_nrt_profile.restype = ctypes.c_int64

        @contextlib.contextmanager
        def _hook(output_dir, device_ids):
            import jax
            jax.devices()
            if device_ids:
                ids = (ctypes.c_int64 * len(device_ids))(*device_ids)
                rc = lib.axon_start_nrt_profile(ids, len(device_ids))
            else:
                rc = lib.axon_start_nrt_profile(None, 0)
            if rc != 0:
                raise RuntimeError(f"axon_start_nrt_profile rc={rc}")
            try:
                yield
            finally:
                n = lib.axon_stop_nrt_profile(str(output_dir).encode())
                print(f"profile: {n} file(s) -> {output_dir}", file=sys.stderr)

        return _hook

    def get_axon_ntff_profile_hook():
        if state["hook"] is None:
            try:
                state["hook"] = _make_hook()
            except OSError:
                state["hook"] = None
        return state["hook"]

    def set_axon_ntff_profile_hook(hook):
        state["hook"] = hook

    mod.get_axon_ntff_profile_hook = get_axon_ntff_profile_hook
    mod.set_axon_ntff_profile_hook = set_axon_ntff_profile_hook
    sys.modules["antenv.axon_hooks"] = mod
    try:
        import antenv
        antenv.axon_hooks = mod
    except ImportError:
        pass


_install_ntff_hook_module()

B, S, E = 2, 1024, 1024
H, HD, TOPK = 16, 64, 64
NCORES = 8
HPC = 4          # heads per core
DL = HPC * HD    # 256 local e dims per core
NT = E // 128    # 8 e-tiles
SCALE = 1.0 / 8.0  # 1/sqrt(hd)
SHIFT = 128.0      # score shift so the 0-sentinel sits far below all kept values

BF16 = ml_dtypes.bfloat16

_CACHE = {}


def _build_nc():
    import concourse.bass as bass
    import concourse.bacc as bacc
    import concourse.tile as tile
    from concourse import mybir

    f32 = mybir.dt.float32
    bf16 = mybir.dt.bfloat16
    AF = mybir.ActivationFunctionType
    OP = mybir.AluOpType

    nc = bacc.Bacc("TRN2", target_bir_lowering=False, debug=False,
                   num_devices=NCORES)

    xT_d = nc.dram_tensor("xT", [E, S], f32, kind="ExternalInput")
    xTb_d = nc.dram_tensor("xTb", [E, S], bf16, kind="ExternalInput")
    wqT_d = nc.dram_tensor("wqT", [E, DL], f32, kind="ExternalInput")
    wkT_d = nc.dram_tensor("wkT", [E, DL], f32, kind="ExternalInput")
    wvT_d = nc.dram_tensor("wvT", [E, DL], bf16, kind="ExternalInput")
    woT_d = nc.dram_tensor("woT", [E, DL], bf16, kind="ExternalInput")
    bq_d = nc.dram_tensor("bq", [DL, 1], f32, kind="ExternalInput")
    bk_d = nc.dram_tensor("bk", [DL, 1], f32, kind="ExternalInput")
    bv_d = nc.dram_tensor("bv", [E, 1], bf16, kind="ExternalInput")
    bo_d = nc.dram_tensor("bo", [1, DL], f32, kind="ExternalInput")
    y_d = nc.dram_tensor("y", [S, DL], f32, kind="ExternalOutput")

    outT_int = [nc.dram_tensor(f"outT_int{i}", [128, S], bf16) for i in range(2)]
    ag_out = [nc.dram_tensor(f"ag_out{i}", [512, S], bf16) for i in range(2)]
    groups = [[0, 1, 2, 3], [4, 5, 6, 7]]

    with tile.TileContext(nc) as tc:
        with tc.tile_pool(name="persist", bufs=1) as pp, \
             tc.tile_pool(name="psum", bufs=1, space="PSUM") as psp:
            qt_sb = [pp.tile([128, S], f32, tag=f"qt{p}", name=f"qtsb{p}")
                     for p in range(2)]
            kt_sb = [pp.tile([128, S], f32, tag=f"kt{p}", name=f"ktsb{p}")
                     for p in range(2)]
            v_sb = pp.tile([128, NT * DL], bf16, tag="v")
            outT_sb = [pp.tile([64, S], bf16, tag=f"ot{h}", name=f"outTsb{h}")
                       for h in range(HPC)]
            ones_sb = pp.tile([1, 128], bf16, tag="ones")
            wo_sb = pp.tile([128, NT * DL], bf16, tag="wo")
            bv_sb = pp.tile([128, NT], bf16, tag="bv")
            bo_sb = pp.tile([1, DL], f32, tag="bo")
            c_sb = pp.tile([1, DL], bf16, tag="c")
            nc.vector.memset(ones_sb[:], 1.0)

            # ------------- phase A: projections -------------
            with tc.tile_pool(name="phaseA", bufs=1) as pa:
                xT_sb = pa.tile([128, NT * S], f32, tag="xT")
                xTb_sb = pa.tile([128, NT * S], bf16, tag="xTb")
                wq_sb = pa.tile([128, NT * DL], f32, tag="wq")
                wk_sb = pa.tile([128, NT * DL], f32, tag="wk")
                wv_sb = pa.tile([128, NT * DL], bf16, tag="wv")
                bq_sb = pa.tile([128, 2], f32, tag="bq")
                bk_sb = pa.tile([128, 2], f32, tag="bk")

                xT_t = xT_d.ap().rearrange("(t p) s -> t p s", p=128)
                xTb_t = xTb_d.ap().rearrange("(t p) s -> t p s", p=128)
                wq_t = wqT_d.ap().rearrange("(t p) d -> t p d", p=128)
                wk_t = wkT_d.ap().rearrange("(t p) d -> t p d", p=128)
                wv_t = wvT_d.ap().rearrange("(t p) d -> t p d", p=128)
                for t in range(NT):
                    nc.sync.dma_start(xT_sb[:, t * S:(t + 1) * S], xT_t[t])
                    nc.sync.dma_start(xTb_sb[:, t * S:(t + 1) * S], xTb_t[t])
                    nc.sync.dma_start(wq_sb[:, t * DL:(t + 1) * DL], wq_t[t])
                    nc.sync.dma_start(wk_sb[:, t * DL:(t + 1) * DL], wk_t[t])
                    nc.sync.dma_start(wv_sb[:, t * DL:(t + 1) * DL], wv_t[t])
                bq_t = bq_d.ap().rearrange("(h p) o -> h p o", p=128)
                bk_t = bk_d.ap().rearrange("(h p) o -> h p o", p=128)
                for p in range(2):
                    nc.sync.dma_start(bq_sb[:, p:p + 1], bq_t[p])
                    nc.sync.dma_start(bk_sb[:, p:p + 1], bk_t[p])

                def qk_proj(p):
                    for (w_sb, b_sb, dst) in ((wk_sb, bk_sb, kt_sb),
                                              (wq_sb, bq_sb, qt_sb)):
                        for nb in range(2):
                            ps = psp.tile([128, 512], f32, tag="small",
                                          bufs=2, name=f"pj{p}{nb}")
                            for t in range(NT):
                                nc.tensor.matmul(
                                    ps[:],
                                    w_sb[:, t * DL + p * 128: t * DL + (p + 1) * 128],
                                    xT_sb[:, t * S + nb * 512: t * S + nb * 512 + 512],
                                    start=(t == 0), stop=(t == NT - 1))
                            nc.scalar.activation(
                                dst[p][:, nb * 512:(nb + 1) * 512], ps[:],
                                AF.Identity, bias=b_sb[:, p:p + 1])

                def v_proj():
                    for kt in range(NT):
                        ps = psp.tile([128, DL], f32, tag="small", bufs=2,
                                      name=f"vp{kt}")
                        for t in range(NT):
                            nc.tensor.matmul(
                                ps[:],
                                xTb_sb[:, t * S + kt * 128: t * S + (kt + 1) * 128],
                                wv_sb[:, t * DL:(t + 1) * DL],
                                start=(t == 0), stop=(t == NT - 1))
                        nc.scalar.activation(v_sb[:, kt * DL:(kt + 1) * DL],
                                             ps[:], AF.Copy)

                qk_proj(0)

                wo_t = woT_d.ap().rearrange("(t p) e -> t p e", p=128)
                bv_t = bv_d.ap().rearrange("(t p) o -> t p o", p=128)
                for t in range(NT):
                    nc.sync.dma_start(wo_sb[:, t * DL:(t + 1) * DL], wo_t[t])
                    nc.sync.dma_start(bv_sb[:, t:t + 1], bv_t[t])
                nc.sync.dma_start(bo_sb[:], bo_d.ap())

                # ------------- phase B: attention per head -------------
                with tc.tile_pool(name="sB", bufs=3) as sB_p, \
                     tc.tile_pool(name="zA", bufs=2) as zA_p, \
                     tc.tile_pool(name="zB", bufs=2) as zB_p, \
                     tc.tile_pool(name="msk", bufs=4) as msk_p, \
                     tc.tile_pool(name="zf", bufs=2) as zB2_p, \
                     tc.tile_pool(name="prob", bufs=2) as prob_p, \
                     tc.tile_pool(name="pmi", bufs=2) as pmi_p, \
                     tc.tile_pool(name="pmw", bufs=3) as pm_p, \
                     tc.tile_pool(name="small", bufs=8) as sm_p, \
                     tc.tile_pool(name="pmT", bufs=1) as pmT_p:
                    for h in range(HPC):
                        pair, sub = h // 2, h % 2
                        r0 = sub * 64
                        if h == 1:
                            qk_proj(1)   # overlap pair-1 proj with head work
                        pmT = pmT_p.tile([128, NT * S], bf16, tag="pmT",
                                         name=f"pmT{h}")
                        for qp in range(NT // 2):
                            if h == 0 and qp == 1:
                                v_proj()   # overlap V projection (PE) with
                                           # head-0 scans (DVE)
                            qts = (2 * qp, 2 * qp + 1)
                            sB_l, z_l, t64_l = [], [], []
                            for qt in qts:
                                sps = psp.tile([128, S], f32, tag="big",
                                               bufs=2, name=f"sps{h}{qt}")
                                for nb in range(2):
                                    nc.tensor.matmul(
                                        sps[:, nb * 512:(nb + 1) * 512],
                                        qt_sb[pair][r0:r0 + 64, qt * 128:(qt + 1) * 128],
                                        kt_sb[pair][r0:r0 + 64, nb * 512:(nb + 1) * 512],
                                        start=True, stop=True)
                                sB = sB_p.tile([128, S], f32, tag="sB",
                                               name=f"sB{h}{qt}")
                                nc.scalar.activation(sB[:], sps[:], AF.Copy,
                                                     bias=SHIFT)
                                sB_l.append(sB)
                                zp = zA_p if qt % 2 == 0 else zB_p
                                z_l.append(zp.tile([128, S], f32, tag="z",
                                                   name=f"z_{h}{qt}"))
                                t64_l.append(sm_p.tile([128, 64], f32,
                                                       tag=f"t64_{qt % 2}",
                                                       name=f"t64_{h}{qt}"))
                            # interleaved 8-round extraction for the two
                            # q-tiles: DVE max8 + in-place match_replace
                            # (as baseline). The FINAL mask (ranks 57-64,
                            # consumed only by ACT exp, never by DVE) runs
                            # as ACT Sign + GpSimd multiply: pure offload.
                            nc.vector.max(t64_l[0][:, 0:8], sB_l[0][:])
                            nc.vector.max(t64_l[1][:, 0:8], sB_l[1][:])
                            nc.vector.match_replace(z_l[0][:], t64_l[0][:, 0:8],
                                                    sB_l[0][:], -1e30)
                            nc.vector.match_replace(z_l[1][:], t64_l[1][:, 0:8],
                                                    sB_l[1][:], -1e30)
                            for r in range(1, 8):
                                nc.vector.max(t64_l[0][:, 8 * r:8 * r + 8],
                                              z_l[0][:])
                                nc.vector.max(t64_l[1][:, 8 * r:8 * r + 8],
                                              z_l[1][:])
                                if r < 7:
                                    nc.vector.match_replace(
                                        z_l[0][:], t64_l[0][:, 8 * r:8 * r + 8],
                                        z_l[0][:], -1e30)
                                    nc.vector.match_replace(
                                        z_l[1][:], t64_l[1][:, 8 * r:8 * r + 8],
                                        z_l[1][:], -1e30)
                            for i, qt in enumerate(qts):
                                t64, sB = t64_l[i], sB_l[i]
                                # final kill of ranks 57..64 (8th in-place
                                # match_replace; consumed only by ACT exp)
                                nc.vector.match_replace(
                                    z_l[i][:], t64[:, 56:64], z_l[i][:],
                                    -1e30)
                                zf = z_l[i][:]
                                e64 = sm_p.tile([128, 64], f32, tag="e64",
                                                name=f"e64_{h}{qt}")
                                den = sm_p.tile([128, 1], f32, tag="den",
                                                name=f"den{h}{qt}")
                                nc.scalar.activation(e64[:], t64[:], AF.Exp,
                                                     scale=SCALE,
                                                     accum_out=den[:])
                                rden = sm_p.tile([128, 1], f32, tag="rden",
                                                 name=f"rden{h}{qt}")
                                nc.vector.reciprocal(rden[:], den[:])
                                # unnormalized exps (shift folds into rden
                                # since den is computed from shifted t64);
                                # no Ln -> no ACT table thrash
                                p_sb = prob_p.tile([128, S], bf16, tag="p",
                                                   name=f"p{h}{qt}")
                                nc.scalar.activation(p_sb[:], sB[:], AF.Exp,
                                                     scale=SCALE)
                                pi_sb = pmi_p.tile([128, S], bf16, tag="pi",
                                                   name=f"pi{h}{qt}")
                                nc.scalar.activation(pi_sb[:], zf, AF.Exp,
                                                     scale=SCALE)
                                pm_sb = pm_p.tile([128, S], bf16, tag="pm",
                                                  name=f"pm{h}{qt}")
                                nc.vector.tensor_tensor(p_sb[:], p_sb[:],
                                                        pi_sb[:],
                                                        op=OP.subtract)
                                nc.vector.tensor_scalar_mul(pm_sb[:], p_sb[:],
                                                            rden[:])
                                for kt in range(NT):
                                    eng = nc.sync
                                    eng.dma_start(
                                        pmT[:, kt * S + qt * 128: kt * S + (qt + 1) * 128],
                                        pm_sb[:, kt * 128:(kt + 1) * 128],
                                        transpose=True)
                        avps = psp.tile([64, S], f32, tag="av", bufs=1,
                                        name=f"avps{h}")
                        for nb in range(2):
                            for kt in range(NT):
                                nc.tensor.matmul(
                                    avps[:, nb * 512:(nb + 1) * 512],
                                    v_sb[:, kt * DL + h * 64: kt * DL + (h + 1) * 64],
                                    pmT[:, kt * S + nb * 512: kt * S + nb * 512 + 512],
                                    start=(kt == 0), stop=(kt == NT - 1))
                        nc.scalar.activation(outT_sb[h][:], avps[:], AF.Copy)
                        nc.sync.dma_start(
                            outT_int[h // 2].ap()[(h % 2) * 64:(h % 2) * 64 + 64, :],
                            outT_sb[h][:])
                        if h % 2 == 1:
                            # AllGather this half; the first one overlaps the
                            # remaining heads' compute
                            nc.gpsimd.collective_compute(
                                "AllGather", mybir.AluOpType.bypass,
                                ins=[outT_int[h // 2].ap()],
                                outs=[ag_out[h // 2].ap()],
                                replica_groups=groups)

            # ------------- phase D: output projection (column-split) -------
            with tc.tile_pool(name="phaseD", bufs=1) as pd, \
                 tc.tile_pool(name="ysb", bufs=2) as yp:
                ot_sb = pd.tile([128, NT * S], bf16, tag="ot")
                for t in [0, 2, 4, 6, 1, 3, 5, 7]:
                    r, half = t // 2, t % 2
                    nc.sync.dma_start(
                        ot_sb[:, t * S:(t + 1) * S],
                        ag_out[half].ap()[r * 128:(r + 1) * 128, :])

                # c = bv @ Wo_local.T + bo_local   (constant row, [1, DL])
                cps = psp.tile([1, DL], f32, tag="small", bufs=2, name="cps")
                for t in range(NT):
                    nc.tensor.matmul(
                        cps[:],
                        bv_sb[:, t:t + 1],
                        wo_sb[:, t * DL:(t + 1) * DL],
                        start=(t == 0), stop=(t == NT - 1))
                nc.vector.tensor_tensor(c_sb[:], cps[:], bo_sb[:], op=OP.add)

                for st in range(NT):
                    y_sb = yp.tile([128, DL], f32, tag="y", name=f"y{st}")
                    yps = psp.tile([128, DL], f32, tag="small", bufs=2,
                                   name=f"yps{st}")
                    for i, t in enumerate([0, 2, 4, 6, 1, 3, 5, 7]):
                        nc.tensor.matmul(
                            yps[:],
                            ot_sb[:, t * S + st * 128: t * S + (st + 1) * 128],
                            wo_sb[:, t * DL:(t + 1) * DL],
                            start=(i == 0), stop=False)
                    nc.tensor.matmul(
                        yps[:],
                        ones_sb[:],
                        c_sb[:],
                        start=False, stop=True)
                    nc.scalar.activation(y_sb[:], yps[:], AF.Copy)
                    nc.sync.dma_start(y_d.ap()[st * 128:(st + 1) * 128, :],
                                      y_sb[:])

    nc.compile()
    return nc


def _get_nc():
    if "nc" not in _CACHE:
        _CACHE["nc"] = _build_nc()
    return _CACHE["nc"]


def _in_maps(x, Wq, bq, Wk, bk, Wv, bv, Wo, bo):
    x = np.asarray(x, np.float32)
    Wq = np.asarray(Wq, np.float32)
    Wk = np.asarray(Wk, np.float32)
    Wv = np.asarray(Wv, np.float32)
    Wo = np.asarray(Wo, np.float32)
    bq = np.asarray(bq, np.float32)
    bk = np.asarray(bk, np.float32)
    bv = np.asarray(bv, np.float32)
    bo = np.asarray(bo, np.float32)

    woT = np.ascontiguousarray(Wo.T)  # [E, E]; cols j = output dims
    bv_r = bv.reshape(E, 1).astype(BF16)
    maps = []
    for c in range(NCORES):
        b = c // 4
        dlo = (c % 4) * DL
        xT = np.ascontiguousarray(x[b].T)
        maps.append({
            "xT": xT,
            "xTb": xT.astype(BF16),
            "wqT": np.ascontiguousarray(Wq[dlo:dlo + DL, :].T),
            "wkT": np.ascontiguousarray(Wk[dlo:dlo + DL, :].T),
            "wvT": np.ascontiguousarray(Wv[dlo:dlo + DL, :].T).astype(BF16),
            "woT": np.ascontiguousarray(woT[:, dlo:dlo + DL]).astype(BF16),
            "bq": np.ascontiguousarray(bq[dlo:dlo + DL].reshape(DL, 1)),
            "bk": np.ascontiguousarray(bk[dlo:dlo + DL].reshape(DL, 1)),
            "bv": bv_r,
            "bo": np.ascontiguousarray(bo[dlo:dlo + DL].reshape(1, DL)),
        })
    return maps


def run_on_hw(inputs, trace=False):
    """Run the bass kernel; returns (output, BassKernelResults)."""
    from concourse.bass_utils import run_bass_kernel_spmd

    nc = _get_nc()
    maps = _in_maps(**inputs)
    res = run_bass_kernel_spmd(nc, maps, core_ids=list(range(NCORES)),
                               trace=trace)
    y = np.empty((B, S, E), np.float32)
    for c in range(NCORES):
        b = c // 4
        dlo = (c % 4) * DL
        y[b][:, dlo:dlo + DL] = np.asarray(res.results[c]["y"])
    return y, res


def kernel(x, Wq, bq, Wk, bk, Wv, bv, Wo, bo):
    y, _ = run_on_hw(dict(x=x, Wq=Wq, bq=bq, Wk=Wk, bk=bk, Wv=Wv, bv=bv,
                          Wo=Wo, bo=bo))
    return y



# revision 5
# speedup vs baseline: 1.0309x; 1.0309x over previous
"""Trainium2 Bass kernel for nn_DynamicSparseAttention (B=2,S=1024,E=1024,H=16,K=64).

Sharding: 8 cores = 2 batches x 4 head-groups (4 heads each).

Per core: QKV projections for its 4 heads (f32 Q/K since top-k selection is
precision-critical), per-head scores, exact top-64 per query row via 8 rounds
of DVE max8 + 7 in-place match_replace (raw scores, no shift). The top-64
mask is applied via one chained tensor_scalar: msk = (sB is_ge t64[:,63]) *
rden, then pm = p * msk where p = exp(sB*scale) is computed on ACT *during*
the scan (off the critical path). den comes from the ACT exp of t64 with
accum_out. V projection and pair-1 Q/K projection overlap the early heads'
scans; Wo/bv loads issue after phase-A x loads. pm transposed with ONE
batched DMA-XBAR transpose per q-tile (3D out AP [128, kt(stride S), 128] --
the ucode transpose writes out[kp, kt, q] = pm[q, kt*128+kp]), AV with V
stationary, AllGather per head-pair (first overlaps remaining compute), and
a column-split output projection split in two passes so only the second-half
e-tiles wait on the final AllGather; each core computes its 256 of 1024
output columns; the host concatenates.

Notes from profiling (throttle-limited device, ~53% util cap): GpSimd
offload of elementwise work is counterproductive -- each Pool op carries
~2us semaphore overhead and the Q7 cores double as collective-compute
cores, so GpSimd tensor_tensor fights the AllGather. ACT Sign/Ln force
~1.3us activation-table reloads per switch. fp32r matmul is TF32-grade
(rel ~1.5e-4): too coarse for scores (selection needs <~1e-5 noise).
is_transpose matmul with a non-identity moving operand produces garbage.
"""

import sys

if "/opt/trn_rl_repo" not in sys.path:
    sys.path.insert(0, "/opt/trn_rl_repo")

import numpy as np
import ml_dtypes


def _install_ntff_hook_module():
    """bass_utils(trace=True) imports antenv.axon_hooks, which this image's
    read-only antenv lacks; provide it via sys.modules (ctypes into
    libaxon_pjrt.so, same recipe as the boot script)."""
    import types, contextlib, ctypes

    if "antenv.axon_hooks" in sys.modules:
        return
    mod = types.ModuleType("antenv.axon_hooks")
    state = {"hook": None}

    def _make_hook(so_path="/opt/axon/libaxon_pjrt.so"):
        lib = ctypes.CDLL(so_path)
        if not hasattr(lib, "axon_start_nrt_profile"):
            return None
        lib.axon_start_nrt_profile.argtypes = [
            ctypes.POINTER(ctypes.c_int64), ctypes.c_size_t]
        lib.axon_start_nrt_profile.restype = ctypes.c_int64
        lib.axon_stop_nrt_profile.argtypes = [ctypes.c_char_p]
        lib.axon_stop_nrt_profile.restype = ctypes.c_int64

        @contextlib.contextmanager
        def _hook(output_dir, device_ids):
            import jax
            jax.devices()
            if device_ids:
                ids = (ctypes.c_int64 * len(device_ids))(*device_ids)
                rc = lib.axon_start_nrt_profile(ids, len(device_ids))
            else:
                rc = lib.axon_start_nrt_profile(None, 0)
            if rc != 0:
                raise RuntimeError(f"axon_start_nrt_profile rc={rc}")
            try:
                yield
            finally:
                n = lib.axon_stop_nrt_profile(str(output_dir).encode())
                print(f"profile: {n} file(s) -> {output_dir}", file=sys.stderr)

        return _hook

    def get_axon_ntff_profile_hook():
        if state["hook"] is None:
            try:
                state["hook"] = _make_hook()
            except OSError:
                state["hook"] = None
        return state["hook"]

    def set_axon_ntff_profile_hook(hook):
        state["hook"] = hook

    mod.get_axon_ntff_profile_hook = get_axon_ntff_profile_hook
    mod.set_axon_ntff_profile_hook = set_axon_ntff_profile_hook
    sys.modules["antenv.axon_hooks"] = mod
    try:
        import antenv
        antenv.axon_hooks = mod
    except ImportError:
        pass


_install_ntff_hook_module()

B, S, E = 2, 1024, 1024
H, HD, TOPK = 16, 64, 64
NCORES = 8
HPC = 4          # heads per core
DL = HPC * HD    # 256 local e dims per core
NT = E // 128    # 8 e-tiles
SCALE = 1.0 / 8.0  # 1/sqrt(hd)

BF16 = ml_dtypes.bfloat16

_CACHE = {}


def _build_nc():
    import concourse.bass as bass
    import concourse.bacc as bacc
    import concourse.tile as tile
    from concourse import mybir

    f32 = mybir.dt.float32
    bf16 = mybir.dt.bfloat16
    AF = mybir.ActivationFunctionType
    OP = mybir.AluOpType

    nc = bacc.Bacc("TRN2", target_bir_lowering=False, debug=False,
                   num_devices=NCORES)

    xT_d = nc.dram_tensor("xT", [E, S], f32, kind="ExternalInput")
    xTb_d = nc.dram_tensor("xTb", [E, S], bf16, kind="ExternalInput")
    wqT_d = nc.dram_tensor("wqT", [E, DL], f32, kind="ExternalInput")
    wkT_d = nc.dram_tensor("wkT", [E, DL], f32, kind="ExternalInput")
    wvT_d = nc.dram_tensor("wvT", [E, DL], bf16, kind="ExternalInput")
    woT_d = nc.dram_tensor("woT", [E, DL], bf16, kind="ExternalInput")
    bq_d = nc.dram_tensor("bq", [DL, 1], f32, kind="ExternalInput")
    bk_d = nc.dram_tensor("bk", [DL, 1], f32, kind="ExternalInput")
    bv_d = nc.dram_tensor("bv", [E, 1], bf16, kind="ExternalInput")
    bo_d = nc.dram_tensor("bo", [1, DL], f32, kind="ExternalInput")
    y_d = nc.dram_tensor("y", [S, DL], f32, kind="ExternalOutput")

    outT_int = [nc.dram_tensor(f"outT_int{i}", [128, S], bf16) for i in range(2)]
    ag_out = [nc.dram_tensor(f"ag_out{i}", [512, S], bf16) for i in range(2)]
    groups = [[0, 1, 2, 3], [4, 5, 6, 7]]

    with tile.TileContext(nc) as tc:
        with tc.tile_pool(name="persist", bufs=1) as pp, \
             tc.tile_pool(name="psum", bufs=1, space="PSUM") as psp:
            qt_sb = [pp.tile([128, S], f32, tag=f"qt{p}", name=f"qtsb{p}")
                     for p in range(2)]
            kt_sb = [pp.tile([128, S], f32, tag=f"kt{p}", name=f"ktsb{p}")
                     for p in range(2)]
            v_sb = pp.tile([128, NT * DL], bf16, tag="v")
            outT_sb = [pp.tile([64, S], bf16, tag=f"ot{h}", name=f"outTsb{h}")
                       for h in range(HPC)]
            ones_sb = pp.tile([1, 128], bf16, tag="ones")
            wo_sb = pp.tile([128, NT * DL], bf16, tag="wo")
            bv_sb = pp.tile([128, NT], bf16, tag="bv")
            bo_sb = pp.tile([1, DL], f32, tag="bo")
            c_sb = pp.tile([1, DL], bf16, tag="c")
            nc.vector.memset(ones_sb[:], 1.0)

            # ------------- phase A: projections -------------
            with tc.tile_pool(name="phaseA", bufs=1) as pa:
                xT_sb = pa.tile([128, NT * S], f32, tag="xT")
                xTb_sb = pa.tile([128, NT * S], bf16, tag="xTb")
                wq_sb = pa.tile([128, NT * DL], f32, tag="wq")
                wk_sb = pa.tile([128, NT * DL], f32, tag="wk")
                wv_sb = pa.tile([128, NT * DL], bf16, tag="wv")
                bq_sb = pa.tile([128, 2], f32, tag="bq")
                bk_sb = pa.tile([128, 2], f32, tag="bk")

                xT_t = xT_d.ap().rearrange("(t p) s -> t p s", p=128)
                xTb_t = xTb_d.ap().rearrange("(t p) s -> t p s", p=128)
                wq_t = wqT_d.ap().rearrange("(t p) d -> t p d", p=128)
                wk_t = wkT_d.ap().rearrange("(t p) d -> t p d", p=128)
                wv_t = wvT_d.ap().rearrange("(t p) d -> t p d", p=128)
                for t in range(NT):
                    nc.sync.dma_start(xT_sb[:, t * S:(t + 1) * S], xT_t[t])
                    nc.sync.dma_start(xTb_sb[:, t * S:(t + 1) * S], xTb_t[t])
                    nc.sync.dma_start(wq_sb[:, t * DL:(t + 1) * DL], wq_t[t])
                    nc.sync.dma_start(wk_sb[:, t * DL:(t + 1) * DL], wk_t[t])
                    nc.sync.dma_start(wv_sb[:, t * DL:(t + 1) * DL], wv_t[t])
                bq_t = bq_d.ap().rearrange("(h p) o -> h p o", p=128)
                bk_t = bk_d.ap().rearrange("(h p) o -> h p o", p=128)
                for p in range(2):
                    nc.sync.dma_start(bq_sb[:, p:p + 1], bq_t[p])
                    nc.sync.dma_start(bk_sb[:, p:p + 1], bk_t[p])

                def qk_proj(p):
                    for (w_sb, b_sb, dst) in ((wk_sb, bk_sb, kt_sb),
                                              (wq_sb, bq_sb, qt_sb)):
                        for nb in range(2):
                            ps = psp.tile([128, 512], f32, tag="small",
                                          bufs=2, name=f"pj{p}{nb}")
                            for t in range(NT):
                                nc.tensor.matmul(
                                    ps[:],
                                    w_sb[:, t * DL + p * 128: t * DL + (p + 1) * 128],
                                    xT_sb[:, t * S + nb * 512: t * S + nb * 512 + 512],
                                    start=(t == 0), stop=(t == NT - 1))
                            nc.scalar.activation(
                                dst[p][:, nb * 512:(nb + 1) * 512], ps[:],
                                AF.Identity, bias=b_sb[:, p:p + 1])

                def v_proj():
                    for kt in range(NT):
                        ps = psp.tile([128, DL], f32, tag="small", bufs=2,
                                      name=f"vp{kt}")
                        for t in range(NT):
                            nc.tensor.matmul(
                                ps[:],
                                xTb_sb[:, t * S + kt * 128: t * S + (kt + 1) * 128],
                                wv_sb[:, t * DL:(t + 1) * DL],
                                start=(t == 0), stop=(t == NT - 1))
                        nc.scalar.activation(v_sb[:, kt * DL:(kt + 1) * DL],
                                             ps[:], AF.Copy)

                qk_proj(0)

                wo_t = woT_d.ap().rearrange("(t p) e -> t p e", p=128)
                bv_t = bv_d.ap().rearrange("(t p) o -> t p o", p=128)
                for t in range(NT):
                    nc.sync.dma_start(wo_sb[:, t * DL:(t + 1) * DL], wo_t[t])
                    nc.sync.dma_start(bv_sb[:, t:t + 1], bv_t[t])
                nc.sync.dma_start(bo_sb[:], bo_d.ap())

                # ------------- phase B: attention per head -------------
                with tc.tile_pool(name="sB", bufs=3) as sB_p, \
                     tc.tile_pool(name="zA", bufs=2) as zA_p, \
                     tc.tile_pool(name="zB", bufs=2) as zB_p, \
                     tc.tile_pool(name="prob", bufs=3) as prob_p, \
                     tc.tile_pool(name="pmi", bufs=2) as pmi_p, \
                     tc.tile_pool(name="pmw", bufs=3) as pm_p, \
                     tc.tile_pool(name="small", bufs=8) as sm_p, \
                     tc.tile_pool(name="pmT", bufs=2) as pmT_p:
                    for h in range(HPC):
                        pair, sub = h // 2, h % 2
                        r0 = sub * 64
                        if h == 1:
                            qk_proj(1)   # overlap pair-1 proj with head work
                        pmT = pmT_p.tile([128, NT * S], bf16, tag="pmT",
                                         name=f"pmT{h}")
                        for qp in range(NT // 2):
                            if h == 0 and qp == 1:
                                v_proj()   # overlap V projection (PE) with
                                           # head-0 scans (DVE)
                            qts = (2 * qp, 2 * qp + 1)
                            sB_l, z_l, t64_l, p_l = [], [], [], []
                            for qt in qts:
                                sps = psp.tile([128, S], f32, tag="big",
                                               bufs=2, name=f"sps{h}{qt}")
                                for nb in range(2):
                                    nc.tensor.matmul(
                                        sps[:, nb * 512:(nb + 1) * 512],
                                        qt_sb[pair][r0:r0 + 64, qt * 128:(qt + 1) * 128],
                                        kt_sb[pair][r0:r0 + 64, nb * 512:(nb + 1) * 512],
                                        start=True, stop=True)
                                sB = sB_p.tile([128, S], f32, tag="sB",
                                               name=f"sB{h}{qt}")
                                nc.scalar.activation(sB[:], sps[:], AF.Copy)
                                sB_l.append(sB)
                                # p = exp(sB*scale) on ACT, issued before the
                                # scan so it runs under the DVE scan
                                p_sb = prob_p.tile([128, S], bf16, tag="p",
                                                   name=f"p{h}{qt}")
                                nc.scalar.activation(p_sb[:], sB[:], AF.Exp,
                                                     scale=SCALE)
                                p_l.append(p_sb)
                                zp = zA_p if qt % 2 == 0 else zB_p
                                z_l.append(zp.tile([128, S], f32, tag="z",
                                                   name=f"z_{h}{qt}"))
                                t64_l.append(sm_p.tile([128, 64], f32,
                                                       tag=f"t64_{qt % 2}",
                                                       name=f"t64_{h}{qt}"))
                            # interleaved extraction for the two q-tiles:
                            # 8 rounds DVE max8, 7 in-place match_replace
                            # (the 8th kill is unnecessary: the top-64 mask
                            # is applied by is_ge against t64[:,63] instead)
                            nc.vector.max(t64_l[0][:, 0:8], sB_l[0][:])
                            nc.vector.max(t64_l[1][:, 0:8], sB_l[1][:])
                            nc.vector.match_replace(z_l[0][:], t64_l[0][:, 0:8],
                                                    sB_l[0][:], -1e30)
                            nc.vector.match_replace(z_l[1][:], t64_l[1][:, 0:8],
                                                    sB_l[1][:], -1e30)
                            for r in range(1, 8):
                                nc.vector.max(t64_l[0][:, 8 * r:8 * r + 8],
                                              z_l[0][:])
                                nc.vector.max(t64_l[1][:, 8 * r:8 * r + 8],
                                              z_l[1][:])
                                if r < 7:
                                    nc.vector.match_replace(
                                        z_l[0][:], t64_l[0][:, 8 * r:8 * r + 8],
                                        z_l[0][:], -1e30)
                                    nc.vector.match_replace(
                                        z_l[1][:], t64_l[1][:, 8 * r:8 * r + 8],
                                        z_l[1][:], -1e30)
                            for i, qt in enumerate(qts):
                                t64, sB, p_sb = t64_l[i], sB_l[i], p_l[i]
                                e64 = sm_p.tile([128, 64], f32, tag="e64",
                                                name=f"e64_{h}{qt}")
                                den = sm_p.tile([128, 1], f32, tag="den",
                                                name=f"den{h}{qt}")
                                nc.scalar.activation(e64[:], t64[:], AF.Exp,
                                                     scale=SCALE,
                                                     accum_out=den[:])
                                rden = sm_p.tile([128, 1], f32, tag="rden",
                                                 name=f"rden{h}{qt}")
                                nc.vector.reciprocal(rden[:], den[:])
                                # msk = (sB >= 64th value) * rden, one chained
                                # tensor_scalar pass; pm = p * msk (bf16 2x)
                                msk = pmi_p.tile([128, S], bf16, tag="pi",
                                                 name=f"msk{h}{qt}")
                                nc.vector.tensor_scalar(
                                    msk[:], sB[:], t64[:, 63:64], rden[:],
                                    op0=OP.is_ge, op1=OP.mult)
                                pm_sb = pm_p.tile([128, S], bf16, tag="pm",
                                                  name=f"pm{h}{qt}")
                                nc.vector.tensor_tensor(pm_sb[:], p_sb[:],
                                                        msk[:], op=OP.mult)
                                # ONE batched XBAR transpose for all 8 kt
                                # blocks: out[kp, kt, q] = pm[q, kt*128+kp]
                                pmT_view = pmT[:].rearrange(
                                    "p (kt s) -> p kt s", kt=NT)[
                                    :, :, qt * 128:(qt + 1) * 128]
                                nc.sync.dma_start(pmT_view, pm_sb[:],
                                                  transpose=True)
                        avps = psp.tile([64, S], f32, tag="av", bufs=1,
                                        name=f"avps{h}")
                        for nb in range(2):
                            for kt in range(NT):
                                nc.tensor.matmul(
                                    avps[:, nb * 512:(nb + 1) * 512],
                                    v_sb[:, kt * DL + h * 64: kt * DL + (h + 1) * 64],
                                    pmT[:, kt * S + nb * 512: kt * S + nb * 512 + 512],
                                    start=(kt == 0), stop=(kt == NT - 1))
                        nc.scalar.activation(outT_sb[h][:], avps[:], AF.Copy)
                        nc.sync.dma_start(
                            outT_int[h // 2].ap()[(h % 2) * 64:(h % 2) * 64 + 64, :],
                            outT_sb[h][:])
                        if h % 2 == 1:
                            # AllGather this half; the first one overlaps the
                            # remaining heads' compute
                            nc.gpsimd.collective_compute(
                                "AllGather", mybir.AluOpType.bypass,
                                ins=[outT_int[h // 2].ap()],
                                outs=[ag_out[h // 2].ap()],
                                replica_groups=groups)

            # ------------- phase D: output projection (column-split) -------
            # Two passes: pass 1 accumulates the first-half e-tiles (ready
            # after AllGather #1, so it overlaps AG #2 on the PE) into SBUF
            # partials; pass 2 adds the second-half e-tiles + bias after
            # AG #2 lands. Only ~5.5us of PE work remains on the tail.
            with tc.tile_pool(name="phaseD", bufs=1) as pd, \
                 tc.tile_pool(name="yhalf", bufs=1) as yhp, \
                 tc.tile_pool(name="ysb", bufs=2) as yp:
                ot_sb = pd.tile([128, NT * S], bf16, tag="ot")
                for t in [0, 2, 4, 6, 1, 3, 5, 7]:
                    r, half = t // 2, t % 2
                    nc.sync.dma_start(
                        ot_sb[:, t * S:(t + 1) * S],
                        ag_out[half].ap()[r * 128:(r + 1) * 128, :])

                # c = bv @ Wo_local.T + bo_local   (constant row, [1, DL])
                cps = psp.tile([1, DL], f32, tag="small", bufs=2, name="cps")
                for t in range(NT):
                    nc.tensor.matmul(
                        cps[:],
                        bv_sb[:, t:t + 1],
                        wo_sb[:, t * DL:(t + 1) * DL],
                        start=(t == 0), stop=(t == NT - 1))
                nc.vector.tensor_tensor(c_sb[:], cps[:], bo_sb[:], op=OP.add)

                yh_sb = yhp.tile([128, NT * DL], f32, tag="yh")
                for st in range(NT):
                    yps = psp.tile([128, DL], f32, tag="small", bufs=2,
                                   name=f"yhps{st}")
                    for i, t in enumerate([0, 2, 4, 6]):
                        nc.tensor.matmul(
                            yps[:],
                            ot_sb[:, t * S + st * 128: t * S + (st + 1) * 128],
                            wo_sb[:, t * DL:(t + 1) * DL],
                            start=(i == 0), stop=False)
                    nc.tensor.matmul(
                        yps[:],
                        ones_sb[:],
                        c_sb[:],
                        start=False, stop=True)
                    nc.scalar.activation(yh_sb[:, st * DL:(st + 1) * DL],
                                         yps[:], AF.Copy)

                for st in range(NT):
                    y_sb = yp.tile([128, DL], f32, tag="y", name=f"y{st}")
                    yps = psp.tile([128, DL], f32, tag="small", bufs=2,
                                   name=f"yps{st}")
                    for i, t in enumerate([1, 3, 5, 7]):
                        nc.tensor.matmul(
                            yps[:],
                            ot_sb[:, t * S + st * 128: t * S + (st + 1) * 128],
                            wo_sb[:, t * DL:(t + 1) * DL],
                            start=(i == 0), stop=(i == 3))
                    nc.vector.tensor_tensor(
                        y_sb[:], yps[:], yh_sb[:, st * DL:(st + 1) * DL],
                        op=OP.add)
                    nc.sync.dma_start(y_d.ap()[st * 128:(st + 1) * 128, :],
                                      y_sb[:])

    nc.compile()
    return nc


def _get_nc():
    if "nc" not in _CACHE:
        _CACHE["nc"] = _build_nc()
    return _CACHE["nc"]


def _in_maps(x, Wq, bq, Wk, bk, Wv, bv, Wo, bo):
    x = np.asarray(x, np.float32)
    Wq = np.asarray(Wq, np.float32)
    Wk = np.asarray(Wk, np.float32)
    Wv = np.asarray(Wv, np.float32)
    Wo = np.asarray(Wo, np.float32)
    bq = np.asarray(bq, np.float32)
    bk = np.asarray(bk, np.float32)
    bv = np.asarray(bv, np.float32)
    bo = np.asarray(bo, np.float32)

    woT = np.ascontiguousarray(Wo.T)  # [E, E]; cols j = output dims
    bv_r = bv.reshape(E, 1).astype(BF16)
    maps = []
    for c in range(NCORES):
        b = c // 4
        dlo = (c % 4) * DL
        xT = np.ascontiguousarray(x[b].T)
        maps.append({
            "xT": xT,
            "xTb": xT.astype(BF16),
            "wqT": np.ascontiguousarray(Wq[dlo:dlo + DL, :].T),
            "wkT": np.ascontiguousarray(Wk[dlo:dlo + DL, :].T),
            "wvT": np.ascontiguousarray(Wv[dlo:dlo + DL, :].T).astype(BF16),
            "woT": np.ascontiguousarray(woT[:, dlo:dlo + DL]).astype(BF16),
            "bq": np.ascontiguousarray(bq[dlo:dlo + DL].reshape(DL, 1)),
            "bk": np.ascontiguousarray(bk[dlo:dlo + DL].reshape(DL, 1)),
            "bv": bv_r,
            "bo": np.ascontiguousarray(bo[dlo:dlo + DL].reshape(1, DL)),
        })
    return maps


def run_on_hw(inputs, trace=False):
    """Run the bass kernel; returns (output, BassKernelResults)."""
    from concourse.bass_utils import run_bass_kernel_spmd

    nc = _get_nc()
    maps = _in_maps(**inputs)
    res = run_bass_kernel_spmd(nc, maps, core_ids=list(range(NCORES)),
                               trace=trace)
    y = np.empty((B, S, E), np.float32)
    for c in range(NCORES):
        b = c // 4
        dlo = (c % 4) * DL
        y[b][:, dlo:dlo + DL] = np.asarray(res.results[c]["y"])
    return y, res


def kernel(x, Wq, bq, Wk, bk, Wv, bv, Wo, bo):
    y, _ = run_on_hw(dict(x=x, Wq=Wq, bq=bq, Wk=Wk, bk=bk, Wv=Wv, bv=bv,
                          Wo=Wo, bo=bo))
    return y



# revision 11
# speedup vs baseline: 1.0962x; 1.0633x over previous
"""Trainium2 Bass kernel for nn_DynamicSparseAttention (B=2,S=1024,E=1024,H=16,K=64).

Sharding: 8 cores = 2 batches x 4 head-groups (4 heads each).

Per core: QKV projections for its 4 heads (f32 Q/K since top-k selection is
precision-critical), per-head scores, exact top-64 per query row via 8 rounds
of DVE max8 + 7 in-place match_replace on raw scores. The scan is interleaved
4 q-tiles wide: match_replace reads the t64 values its max8 wrote, and the
DVE write-back latency after a max8 completes is ~1.8us -- with only 2-way
interleave every match_replace stalled ~1.1us (200us total). 4-way gives
~2.1us of separation and removes the stall. The top-64 mask is one chained
tensor_scalar: msk = (sB is_ge t64[:,63]) * rden, then pm = p * msk where
p = exp(sB*scale) runs on ACT during the scan. den comes from the ACT exp of
t64 with accum_out; one reciprocal per 4-tile group (den packed [128,4]).
pm is transposed with ONE batched DMA-XBAR transpose per q-tile (3D out AP
[128, kt(stride S), 128]; out[kp,kt,q] = pm[q, kt*128+kp]) -- the per-call
cost is per-instruction overhead, so batching 8 tiles into one call took the
transpose engine time from 319us to 40us. AV runs per 512-wide q-half as
soon as that half's 4 transposes land. AllGather is split 3+1 heads (e-dims
host-permuted so gathered rows stay 128-aligned): AG#1 (h0-h2) fires after
head 2 and hides under head-3 compute; only the small AG#2 (h3) plus 2 of 8
output-projection e-tiles sit on the tail. Phase A pools are closed after
pair-1 projections to make SBUF room for the wider scan pipeline.

Notes from profiling (throttle-limited device, ~53% util cap): GpSimd
offload of elementwise work is counterproductive -- each Pool op carries
~2us semaphore overhead and the Q7 cores double as collective-compute
cores, so GpSimd tensor_tensor fights the AllGather. ACT Sign/Ln force
~1.3us activation-table reloads per switch. fp32r matmul is TF32-grade
(rel ~1.5e-4): too coarse for scores (selection needs <~1e-5 noise).
is_transpose matmul with a non-identity moving operand produces garbage.
DVE per-op dispatch overhead makes hierarchical (chunked) top-k scans a
wash vs straight 8-round extraction.
"""

import contextlib
import sys

if "/opt/trn_rl_repo" not in sys.path:
    sys.path.insert(0, "/opt/trn_rl_repo")

import numpy as np
import ml_dtypes


def _install_ntff_hook_module():
    """bass_utils(trace=True) imports antenv.axon_hooks, which this image's
    read-only antenv lacks; provide it via sys.modules (ctypes into
    libaxon_pjrt.so, same recipe as the boot script)."""
    import types, contextlib, ctypes

    if "antenv.axon_hooks" in sys.modules:
        return
    mod = types.ModuleType("antenv.axon_hooks")
    state = {"hook": None}

    def _make_hook(so_path="/opt/axon/libaxon_pjrt.so"):
        lib = ctypes.CDLL(so_path)
        if not hasattr(lib, "axon_start_nrt_profile"):
            return None
        lib.axon_start_nrt_profile.argtypes = [
            ctypes.POINTER(ctypes.c_int64), ctypes.c_size_t]
        lib.axon_start_nrt_profile.restype = ctypes.c_int64
        lib.axon_stop_nrt_profile.argtypes = [ctypes.c_char_p]
        lib.axon_stop_nrt_profile.restype = ctypes.c_int64

        @contextlib.contextmanager
        def _hook(output_dir, device_ids):
            import jax
            jax.devices()
            if device_ids:
                ids = (ctypes.c_int64 * len(device_ids))(*device_ids)
                rc = lib.axon_start_nrt_profile(ids, len(device_ids))
            else:
                rc = lib.axon_start_nrt_profile(None, 0)
            if rc != 0:
                raise RuntimeError(f"axon_start_nrt_profile rc={rc}")
            try:
                yield
            finally:
                n = lib.axon_stop_nrt_profile(str(output_dir).encode())
                print(f"profile: {n} file(s) -> {output_dir}", file=sys.stderr)

        return _hook

    def get_axon_ntff_profile_hook():
        if state["hook"] is None:
            try:
                state["hook"] = _make_hook()
            except OSError:
                state["hook"] = None
        return state["hook"]

    def set_axon_ntff_profile_hook(hook):
        state["hook"] = hook

    mod.get_axon_ntff_profile_hook = get_axon_ntff_profile_hook
    mod.set_axon_ntff_profile_hook = set_axon_ntff_profile_hook
    sys.modules["antenv.axon_hooks"] = mod
    try:
        import antenv
        antenv.axon_hooks = mod
    except ImportError:
        pass


_install_ntff_hook_module()

B, S, E = 2, 1024, 1024
H, HD, TOPK = 16, 64, 64
NCORES = 8
HPC = 4          # heads per core
DL = HPC * HD    # 256 local e dims per core
NT = E // 128    # 8 e-tiles
SCALE = 1.0 / 8.0  # 1/sqrt(hd)

BF16 = ml_dtypes.bfloat16

_CACHE = {}


def _e_perm():
    """Contraction-dim permutation matching the 3+1 head AllGather split:
    ag0 rows = [core c: h0,h1,h2] (192 each), ag1 rows = [core c: h3]."""
    perm = []
    for cc in range(4):
        for hh in range(3):
            perm.extend(range(cc * 256 + hh * 64, cc * 256 + hh * 64 + 64))
    for cc in range(4):
        perm.extend(range(cc * 256 + 192, cc * 256 + 256))
    return np.asarray(perm, np.int64)


def _build_nc():
    import concourse.bass as bass
    import concourse.bacc as bacc
    import concourse.tile as tile
    from concourse import mybir

    f32 = mybir.dt.float32
    bf16 = mybir.dt.bfloat16
    AF = mybir.ActivationFunctionType
    OP = mybir.AluOpType

    nc = bacc.Bacc("TRN2", target_bir_lowering=False, debug=False,
                   num_devices=NCORES)

    xT_d = nc.dram_tensor("xT", [E, S], f32, kind="ExternalInput")
    xTb_d = nc.dram_tensor("xTb", [E, S], bf16, kind="ExternalInput")
    wqT_d = nc.dram_tensor("wqT", [E, DL], f32, kind="ExternalInput")
    wkT_d = nc.dram_tensor("wkT", [E, DL], f32, kind="ExternalInput")
    wvT_d = nc.dram_tensor("wvT", [E, DL], bf16, kind="ExternalInput")
    woT_d = nc.dram_tensor("woT", [E, DL], bf16, kind="ExternalInput")
    bq_d = nc.dram_tensor("bq", [DL, 1], f32, kind="ExternalInput")
    bk_d = nc.dram_tensor("bk", [DL, 1], f32, kind="ExternalInput")
    bv_d = nc.dram_tensor("bv", [E, 1], bf16, kind="ExternalInput")
    bo_d = nc.dram_tensor("bo", [1, DL], f32, kind="ExternalInput")
    y_d = nc.dram_tensor("y", [S, DL], f32, kind="ExternalOutput")

    outT_int = [nc.dram_tensor("outT_int0", [192, S], bf16),
                nc.dram_tensor("outT_int1", [64, S], bf16)]
    ag_out = [nc.dram_tensor("ag_out0", [768, S], bf16),
              nc.dram_tensor("ag_out1", [256, S], bf16)]
    groups = [[0, 1, 2, 3], [4, 5, 6, 7]]

    with tile.TileContext(nc) as tc:
        with tc.tile_pool(name="persist", bufs=1) as pp, \
             tc.tile_pool(name="psum", bufs=1, space="PSUM") as psp:
            qt_sb = [pp.tile([128, S], f32, tag=f"qt{p}", name=f"qtsb{p}")
                     for p in range(2)]
            kt_sb = [pp.tile([128, S], f32, tag=f"kt{p}", name=f"ktsb{p}")
                     for p in range(2)]
            v_sb = pp.tile([128, NT * DL], bf16, tag="v")
            outT_sb = [pp.tile([64, S], bf16, tag=f"ot{h}", name=f"outTsb{h}")
                       for h in range(HPC)]
            ones_sb = pp.tile([1, 128], bf16, tag="ones")
            wo_sb = pp.tile([128, NT * DL], bf16, tag="wo")
            bv_sb = pp.tile([128, NT], bf16, tag="bv")
            bo_sb = pp.tile([1, DL], f32, tag="bo")
            c_sb = pp.tile([1, DL], bf16, tag="c")
            nc.vector.memset(ones_sb[:], 1.0)

            # ------------- phase A: projections -------------
            stackA = contextlib.ExitStack()
            pa = stackA.enter_context(tc.tile_pool(name="phaseA", bufs=1))
            xT_sb = pa.tile([128, NT * S], f32, tag="xT")
            xTb_sb = pa.tile([128, NT * S], bf16, tag="xTb")
            wq_sb = pa.tile([128, NT * DL], f32, tag="wq")
            wk_sb = pa.tile([128, NT * DL], f32, tag="wk")
            wv_sb = pa.tile([128, NT * DL], bf16, tag="wv")
            bq_sb = pa.tile([128, 2], f32, tag="bq")
            bk_sb = pa.tile([128, 2], f32, tag="bk")

            xT_t = xT_d.ap().rearrange("(t p) s -> t p s", p=128)
            xTb_t = xTb_d.ap().rearrange("(t p) s -> t p s", p=128)
            wq_t = wqT_d.ap().rearrange("(t p) d -> t p d", p=128)
            wk_t = wkT_d.ap().rearrange("(t p) d -> t p d", p=128)
            wv_t = wvT_d.ap().rearrange("(t p) d -> t p d", p=128)
            bq_t = bq_d.ap().rearrange("(h p) o -> h p o", p=128)
            bk_t = bk_d.ap().rearrange("(h p) o -> h p o", p=128)
            for p in range(2):
                nc.sync.dma_start(bq_sb[:, p:p + 1], bq_t[p])
                nc.sync.dma_start(bk_sb[:, p:p + 1], bk_t[p])
            # pair-0 critical loads first: x + pair-0 columns of Wq/Wk
            for t in range(NT):
                nc.sync.dma_start(xT_sb[:, t * S:(t + 1) * S], xT_t[t])
                nc.sync.dma_start(wk_sb[:, t * DL:t * DL + 128],
                                  wk_t[t][:, 0:128])
                nc.sync.dma_start(wq_sb[:, t * DL:t * DL + 128],
                                  wq_t[t][:, 0:128])
            for t in range(NT):
                nc.sync.dma_start(xTb_sb[:, t * S:(t + 1) * S], xTb_t[t])
                nc.sync.dma_start(wv_sb[:, t * DL:(t + 1) * DL], wv_t[t])
                nc.sync.dma_start(wk_sb[:, t * DL + 128:(t + 1) * DL],
                                  wk_t[t][:, 128:256])
                nc.sync.dma_start(wq_sb[:, t * DL + 128:(t + 1) * DL],
                                  wq_t[t][:, 128:256])

            def qk_proj(p):
                for (w_sb, b_sb, dst) in ((wk_sb, bk_sb, kt_sb),
                                          (wq_sb, bq_sb, qt_sb)):
                    for nb in range(2):
                        ps = psp.tile([128, 512], f32, tag="small",
                                      bufs=2, name=f"pj{p}{nb}")
                        for t in range(NT):
                            nc.tensor.matmul(
                                ps[:],
                                w_sb[:, t * DL + p * 128: t * DL + (p + 1) * 128],
                                xT_sb[:, t * S + nb * 512: t * S + nb * 512 + 512],
                                start=(t == 0), stop=(t == NT - 1))
                        nc.scalar.activation(
                            dst[p][:, nb * 512:(nb + 1) * 512], ps[:],
                            AF.Identity, bias=b_sb[:, p:p + 1])

            def v_proj():
                for kt in range(NT):
                    ps = psp.tile([128, DL], f32, tag="small", bufs=2,
                                  name=f"vp{kt}")
                    for t in range(NT):
                        nc.tensor.matmul(
                            ps[:],
                            xTb_sb[:, t * S + kt * 128: t * S + (kt + 1) * 128],
                            wv_sb[:, t * DL:(t + 1) * DL],
                            start=(t == 0), stop=(t == NT - 1))
                    nc.scalar.activation(v_sb[:, kt * DL:(kt + 1) * DL],
                                         ps[:], AF.Copy)

            qk_proj(0)

            wo_t = woT_d.ap().rearrange("(t p) e -> t p e", p=128)
            bv_t = bv_d.ap().rearrange("(t p) o -> t p o", p=128)
            for t in range(NT):
                nc.sync.dma_start(wo_sb[:, t * DL:(t + 1) * DL], wo_t[t])
                nc.sync.dma_start(bv_sb[:, t:t + 1], bv_t[t])
            nc.sync.dma_start(bo_sb[:], bo_d.ap())

            # ------------- phase B: attention per head -------------
            with tc.tile_pool(name="sB", bufs=6) as sB_p, \
                 tc.tile_pool(name="zz", bufs=4) as z_p, \
                 tc.tile_pool(name="prob", bufs=6) as prob_p, \
                 tc.tile_pool(name="pmi", bufs=3) as pmi_p, \
                 tc.tile_pool(name="pmw", bufs=4) as pm_p, \
                 tc.tile_pool(name="small", bufs=6) as sm_p, \
                 tc.tile_pool(name="pmT", bufs=1) as pmT_p:
                for h in range(HPC):
                    pair, sub = h // 2, h % 2
                    r0 = sub * 64
                    pmT = pmT_p.tile([128, NT * S], bf16, tag="pmT",
                                     name=f"pmT{h}")
                    for g in range(2):   # two groups of 4 q-tiles
                        qts = range(4 * g, 4 * g + 4)
                        sB_l, z_l, t64_l, p_l = [], [], [], []
                        den = sm_p.tile([128, 4], f32, tag=f"den{g}",
                                        name=f"den{h}{g}")
                        rden = sm_p.tile([128, 4], f32, tag=f"rden{g}",
                                         name=f"rden{h}{g}")
                        for qt in qts:
                            sps = psp.tile([128, S], f32, tag="big",
                                           bufs=2, name=f"sps{h}{qt}")
                            for nb in range(2):
                                nc.tensor.matmul(
                                    sps[:, nb * 512:(nb + 1) * 512],
                                    qt_sb[pair][r0:r0 + 64, qt * 128:(qt + 1) * 128],
                                    kt_sb[pair][r0:r0 + 64, nb * 512:(nb + 1) * 512],
                                    start=True, stop=True)
                            sB = sB_p.tile([128, S], f32, tag="sB",
                                           name=f"sB{h}{qt}")
                            nc.scalar.activation(sB[:], sps[:], AF.Copy)
                            sB_l.append(sB)
                            # p = exp(sB*scale) on ACT, during the DVE scan
                            p_sb = prob_p.tile([128, S], bf16, tag="p",
                                               name=f"p{h}{qt}")
                            nc.scalar.activation(p_sb[:], sB[:], AF.Exp,
                                                 scale=SCALE)
                            p_l.append(p_sb)
                            z_l.append(z_p.tile([128, S], f32, tag="z",
                                                name=f"z_{h}{qt}"))
                            t64_l.append(sm_p.tile([128, 64], f32,
                                                   tag=f"t64_{qt % 4}",
                                                   name=f"t64_{h}{qt}"))
                        # 4-way interleaved extraction: 8 rounds max8, 7
                        # match_replace. >=3 ops separate each max8 from the
                        # match_replace that reads its t64 output, covering
                        # the ~1.8us DVE write-back latency. The 8th kill is
                        # unneeded: the mask is is_ge against t64[:,63].
                        for i in range(4):
                            nc.vector.max(t64_l[i][:, 0:8], sB_l[i][:])
                        for i in range(4):
                            nc.vector.match_replace(
                                z_l[i][:], t64_l[i][:, 0:8], sB_l[i][:],
                                -1e30)
                        for r in range(1, 8):
                            for i in range(4):
                                nc.vector.max(t64_l[i][:, 8 * r:8 * r + 8],
                                              z_l[i][:])
                            if r < 7:
                                for i in range(4):
                                    nc.vector.match_replace(
                                        z_l[i][:],
                                        t64_l[i][:, 8 * r:8 * r + 8],
                                        z_l[i][:], -1e30)
                        for i, qt in enumerate(qts):
                            e64 = sm_p.tile([128, 64], f32, tag=f"e64_{i}",
                                            name=f"e64_{h}{qt}")
                            nc.scalar.activation(e64[:], t64_l[i][:], AF.Exp,
                                                 scale=SCALE,
                                                 accum_out=den[:, i:i + 1])
                        nc.vector.reciprocal(rden[:], den[:])
                        for i, qt in enumerate(qts):
                            t64, sB, p_sb = t64_l[i], sB_l[i], p_l[i]
                            # msk = (sB >= 64th value) * rden, one chained
                            # tensor_scalar pass; pm = p * msk (bf16 2x)
                            msk = pmi_p.tile([128, S], bf16, tag="pi",
                                             name=f"msk{h}{qt}")
                            nc.vector.tensor_scalar(
                                msk[:], sB[:], t64[:, 63:64],
                                rden[:, i:i + 1],
                                op0=OP.is_ge, op1=OP.mult)
                            pm_sb = pm_p.tile([128, S], bf16, tag="pm",
                                              name=f"pm{h}{qt}")
                            nc.vector.tensor_tensor(pm_sb[:], p_sb[:],
                                                    msk[:], op=OP.mult)
                            # ONE batched XBAR transpose for all 8 kt
                            # blocks: out[kp, kt, q] = pm[q, kt*128+kp]
                            pmT_view = pmT[:].rearrange(
                                "p (kt s) -> p kt s", kt=NT)[
                                :, :, qt * 128:(qt + 1) * 128]
                            nc.sync.dma_start(pmT_view, pm_sb[:],
                                              transpose=True)
                        if h == 0 and g == 0:
                            v_proj()   # on PE during the head-0 scans,
                                       # before the first AV consumes v_sb
                        # AV for this 512-wide q-half (needs only this
                        # group's transposes)
                        avps = psp.tile([64, 512], f32, tag="av", bufs=2,
                                        name=f"avps{h}{g}")
                        for kt in range(NT):
                            nc.tensor.matmul(
                                avps[:],
                                v_sb[:, kt * DL + h * 64: kt * DL + (h + 1) * 64],
                                pmT[:, kt * S + g * 512: kt * S + g * 512 + 512],
                                start=(kt == 0), stop=(kt == NT - 1))
                        nc.scalar.activation(
                            outT_sb[h][:, g * 512:(g + 1) * 512], avps[:],
                            AF.Copy)
                    if h == 0:
                        qk_proj(1)   # runs on PE during head-0/1 scans
                    if h < 3:
                        nc.sync.dma_start(
                            outT_int[0].ap()[h * 64:(h + 1) * 64, :],
                            outT_sb[h][:])
                    else:
                        nc.sync.dma_start(outT_int[1].ap()[0:64, :],
                                          outT_sb[h][:])
                    if h == 2:
                        # AllGather heads 0-2; overlaps head-3 compute
                        nc.gpsimd.collective_compute(
                            "AllGather", mybir.AluOpType.bypass,
                            ins=[outT_int[0].ap()],
                            outs=[ag_out[0].ap()],
                            replica_groups=groups)
                    if h == 3:
                        nc.gpsimd.collective_compute(
                            "AllGather", mybir.AluOpType.bypass,
                            ins=[outT_int[1].ap()],
                            outs=[ag_out[1].ap()],
                            replica_groups=groups)

            stackA.close()   # release phase-A SBUF before phase D

            # ------------- phase D: output projection (column-split) -------
            # e-dims are host-permuted so ag_out0 covers e-tiles 0..5 and
            # ag_out1 covers e-tiles 6..7. Pass 1 (t 0..5 + bias) depends
            # only on AG#1 and overlaps AG#2; pass 2 (t 6..7) runs after.
            with tc.tile_pool(name="phaseD", bufs=1) as pd, \
                 tc.tile_pool(name="yhalf", bufs=1) as yhp, \
                 tc.tile_pool(name="ysb", bufs=2) as yp:
                ot_sb = pd.tile([128, NT * S], bf16, tag="ot")
                for t in range(6):
                    nc.sync.dma_start(
                        ot_sb[:, t * S:(t + 1) * S],
                        ag_out[0].ap()[t * 128:(t + 1) * 128, :])
                for t in (6, 7):
                    nc.sync.dma_start(
                        ot_sb[:, t * S:(t + 1) * S],
                        ag_out[1].ap()[(t - 6) * 128:(t - 5) * 128, :])

                # c = bv @ Wo_local.T + bo_local   (constant row, [1, DL])
                cps = psp.tile([1, DL], f32, tag="small", bufs=2, name="cps")
                for t in range(NT):
                    nc.tensor.matmul(
                        cps[:],
                        bv_sb[:, t:t + 1],
                        wo_sb[:, t * DL:(t + 1) * DL],
                        start=(t == 0), stop=(t == NT - 1))
                nc.vector.tensor_tensor(c_sb[:], cps[:], bo_sb[:], op=OP.add)

                yh_sb = yhp.tile([128, NT * DL], f32, tag="yh")
                for st in range(NT):
                    yps = psp.tile([128, DL], f32, tag="small", bufs=2,
                                   name=f"yhps{st}")
                    for i, t in enumerate(range(6)):
                        nc.tensor.matmul(
                            yps[:],
                            ot_sb[:, t * S + st * 128: t * S + (st + 1) * 128],
                            wo_sb[:, t * DL:(t + 1) * DL],
                            start=(i == 0), stop=False)
                    nc.tensor.matmul(
                        yps[:],
                        ones_sb[:],
                        c_sb[:],
                        start=False, stop=True)
                    nc.scalar.activation(yh_sb[:, st * DL:(st + 1) * DL],
                                         yps[:], AF.Copy)

                for st in range(NT):
                    y_sb = yp.tile([128, DL], f32, tag="y", name=f"y{st}")
                    yps = psp.tile([128, DL], f32, tag="small", bufs=2,
                                   name=f"yps{st}")
                    for i, t in enumerate((6, 7)):
                        nc.tensor.matmul(
                            yps[:],
                            ot_sb[:, t * S + st * 128: t * S + (st + 1) * 128],
                            wo_sb[:, t * DL:(t + 1) * DL],
                            start=(i == 0), stop=(i == 1))
                    nc.vector.tensor_tensor(
                        y_sb[:], yps[:], yh_sb[:, st * DL:(st + 1) * DL],
                        op=OP.add)
                    nc.sync.dma_start(y_d.ap()[st * 128:(st + 1) * 128, :],
                                      y_sb[:])

    nc.compile()
    return nc


def _get_nc():
    if "nc" not in _CACHE:
        _CACHE["nc"] = _build_nc()
    return _CACHE["nc"]


def _in_maps(x, Wq, bq, Wk, bk, Wv, bv, Wo, bo):
    x = np.asarray(x, np.float32)
    Wq = np.asarray(Wq, np.float32)
    Wk = np.asarray(Wk, np.float32)
    Wv = np.asarray(Wv, np.float32)
    Wo = np.asarray(Wo, np.float32)
    bq = np.asarray(bq, np.float32)
    bk = np.asarray(bk, np.float32)
    bv = np.asarray(bv, np.float32)
    bo = np.asarray(bo, np.float32)

    perm = _e_perm()
    woT = np.ascontiguousarray(Wo.T)[perm, :]  # [E, E] rows in ag order
    bv_r = bv[perm].reshape(E, 1).astype(BF16)
    maps = []
    for c in range(NCORES):
        b = c // 4
        dlo = (c % 4) * DL
        xT = np.ascontiguousarray(x[b].T)
        maps.append({
            "xT": xT,
            "xTb": xT.astype(BF16),
            "wqT": np.ascontiguousarray(Wq[dlo:dlo + DL, :].T),
            "wkT": np.ascontiguousarray(Wk[dlo:dlo + DL, :].T),
            "wvT": np.ascontiguousarray(Wv[dlo:dlo + DL, :].T).astype(BF16),
            "woT": np.ascontiguousarray(woT[:, dlo:dlo + DL]).astype(BF16),
            "bq": np.ascontiguousarray(bq[dlo:dlo + DL].reshape(DL, 1)),
            "bk": np.ascontiguousarray(bk[dlo:dlo + DL].reshape(DL, 1)),
            "bv": bv_r,
            "bo": np.ascontiguousarray(bo[dlo:dlo + DL].reshape(1, DL)),
        })
    return maps


def run_on_hw(inputs, trace=False):
    """Run the bass kernel; returns (output, BassKernelResults)."""
    from concourse.bass_utils import run_bass_kernel_spmd

    nc = _get_nc()
    maps = _in_maps(**inputs)
    res = run_bass_kernel_spmd(nc, maps, core_ids=list(range(NCORES)),
                               trace=trace)
    y = np.empty((B, S, E), np.float32)
    for c in range(NCORES):
        b = c // 4
        dlo = (c % 4) * DL
        y[b][:, dlo:dlo + DL] = np.asarray(res.results[c]["y"])
    return y, res


def kernel(x, Wq, bq, Wk, bk, Wv, bv, Wo, bo):
    y, _ = run_on_hw(dict(x=x, Wq=Wq, bq=bq, Wk=Wk, bk=bk, Wv=Wv, bv=bv,
                          Wo=Wo, bo=bo))
    return y
